# revision 36
# baseline (speedup 1.0000x reference)
"""AttentivePool (B=16, S=8192, H=768, nH=12, Dh=64, Q=1) for 8 Trainium2 NeuronCores.

Strategy (data-parallel over batch: 2 batches per core):
  Since Q == 1, the K projection collapses to a single 12x768 matrix
  C[h,:] = sum_d q[h,d] * w_k[h*64+d,:] / sqrt(64), so
  scores[b,h,s] = x[b,s,:] . C[h,:]   (b_k adds a per-head constant -> softmax invariant).
  The V/output projections commute with the softmax-weighted sum over s:
  out[b] = w_out_gated @ blockdiag(w_v) @ (attn-weighted mean of x) + const.
  Per batch the device computes:
    sigma = C @ x^T            (PE, needs x^T: k on partitions)
    p     = exp(sigma - m_h)   (ACT; accum_out gives l = sum_s p for free)
    acc   = p^T . x            (PE, needs natural x: s on partitions)
  v2 over the baseline:
    * sigma is 2-col-group packed (j-tiles 0-2 in array cols 0-31, 3-5 in
      32-63, concurrent) with a DVE add combining the two partials.
    * acc is 4-col-group packed: the 4 pooled subtiles of each chunk run
      concurrently in array col groups 0-3, PSUM partitions 32g..32g+12.
    * TCH of every 4 512-chunks get x^T built ON-CHIP from the natural
      stream (PE transpose vs a 128x128 identity + PSUM->SBUF copies on
      DVE/ACT/GPSIMD), cutting the HBM x^T stream by TCH/4.
    * finalize sums the 4 acc groups inside row-tiled accumulating
      transpose matmuls; the 1/l scaling folds into the s1 projection copy.
  Host prep: layout/dtype transforms + exact fold of gate/biases.
"""

import os
import sys
import types

import numpy as np

B, S, H = 16, 8192, 768
NH, DH = 12, 64
NCORES = 8
BPC = B // NCORES          # batches per core
CHUNK = 512                # scores chunk (s columns per PSUM tile)
DMACHUNK = 2048            # DMA granularity in s
NCH = S // CHUNK           # 16 chunks per batch
NSUB = CHUNK // 128        # 4 pooled subtiles per chunk
KT = H // 128              # 6 k-tiles
OPC = DMACHUNK // CHUNK    # 4 512-chunks per DMA chunk

TCH = int(os.environ.get("KERN_TCH", "2"))  # 512-chunks per DMA chunk transposed on-chip (0..4)
KEEP = OPC - TCH           # 512-chunks per DMA chunk streamed as x^T from HBM
XT8 = bool(int(os.environ.get("KERN_XT8", "0")))  # stream the HBM x^T portion as fp8e4m3

F16 = np.float16
F32 = np.float32


def _split_sem_waits(nc, mybir, max_waits=1):
    """walrus codegen rejects >1 semaphore wait per instruction; spread extras
    over preceding same-engine NoOps."""
    for f in nc.m.functions:
        for blk in f.blocks:
            insts = blk.instructions
            new = []
            for inst in insts:
                si = inst.sync_info
                waits = list(si.on_wait) if (si and si.on_wait) else []
                if len(waits) > max_waits:
                    upd = list(si.on_update) if si.on_update else []
                    chunks = [waits[i:i + max_waits] for i in range(0, len(waits), max_waits)]
                    for ci, ch in enumerate(chunks[:-1]):
                        nop = mybir.InstNoOp(name=f"{inst.name}-wsplit{ci}")
                        nop.engine = inst.engine
                        nop.sync_info = mybir.SyncInfo(on_wait=ch, on_update=[])
                        new.append(nop)
                    inst.sync_info = mybir.SyncInfo(on_wait=chunks[-1], on_update=upd)
                new.append(inst)
            blk.instructions = new


def _build_nc(num_devices=NCORES, split_waits=True):
    import concourse.bass as bass
    import concourse.tile as tile
    import concourse.mybir as mybir

    f16 = mybir.dt.float16
    f32 = mybir.dt.float32
    f8 = mybir.dt.float8e4
    xtdt = f8 if XT8 else f16

    nc = bass.Bass("TRN2", target_bir_lowering=False, debug=False,
                   num_devices=num_devices)

    xth_d = xn_d = None
    if KEEP:
        xth_d = nc.dram_tensor("xth", (BPC, S // DMACHUNK, 128, KEEP, KT, CHUNK),
                               xtdt, kind="ExternalInput").ap()
    xn_d = nc.dram_tensor("xn", (BPC, S // DMACHUNK, 128, DMACHUNK // 128, H),
                          f16, kind="ExternalInput").ap()
    ct_d = nc.dram_tensor("ct", (H, NH), f16, kind="ExternalInput").ap()
    mh_d = nc.dram_tensor("mh", (NH, BPC), f32, kind="ExternalInput").ap()
    wvt_d = nc.dram_tensor("wvt", (H, H), f16, kind="ExternalInput").ap()
    wog_d = nc.dram_tensor("wog", (H, H), f16, kind="ExternalInput").ap()
    b2_d = nc.dram_tensor("b2", (1, H), f32, kind="ExternalInput").ap()
    id16_d = nc.dram_tensor("id16", (NH, NH), f16, kind="ExternalInput").ap()
    id32x4_d = nc.dram_tensor("id32x4", (128, NH), f32, kind="ExternalInput").ap()
    id128_d = nc.dram_tensor("id128", (128, 128), f16, kind="ExternalInput").ap()
    out_d = nc.dram_tensor("out", (BPC, H), f32, kind="ExternalOutput").ap()

    with tile.TileContext(nc) as tc:
        with tc.tile_pool(name="consts", bufs=1) as consts, \
             tc.tile_pool(name="xpool", bufs=2) as xpool, \
             tc.tile_pool(name="spool", bufs=4) as spool, \
             tc.tile_pool(name="apool", bufs=2) as apool, \
             tc.tile_pool(name="ps_scr", bufs=2, space="PSUM") as ps_scr, \
             tc.tile_pool(name="ps_acc", bufs=2, space="PSUM") as ps_acc:

            # ---- load constants (ct first: it gates the first matmul) ----
            ct_sb = consts.tile([128, KT, NH], f16, tag="ct")
            nc.sync.dma_start(out=ct_sb,
                              in_=ct_d.rearrange("(t p) h -> p t h", p=128))
            id16_sb = consts.tile([NH, NH], f16, tag="id16")
            nc.scalar.dma_start(out=id16_sb, in_=id16_d)
            id128_sb = consts.tile([128, 128], f16, tag="id128")
            nc.scalar.dma_start(out=id128_sb, in_=id128_d)
            mh_sb = consts.tile([NH, BPC], f32, tag="mh")
            nc.scalar.dma_start(out=mh_sb, in_=mh_d)
            id32x4_sb = consts.tile([128, NH], f32, tag="id32x4")
            nc.scalar.dma_start(out=id32x4_sb, in_=id32x4_d)

            pooledT_sb = consts.tile([128, KT, 2 * NH], f16, tag="pooledT")  # col = 2h+b per k-tile

            # projection weights: DMA'd mid-way through batch 0 (ACT ring)
            wv_sb = consts.tile([128, KT, H], f16, tag="wv")
            wog_sb = [consts.tile([128, H], f16, tag=f"wog{t}", name=f"wog_sb{t}")
                      for t in range(KT)]
            b2_sb = consts.tile([1, H], f32, tag="b2")
            o_sb = [consts.tile([128, BPC], f16, tag=f"o{t}", name=f"o_sb{t}")
                    for t in range(KT)]

            _oT = [None]
            _rls = {}
            laccs = []
            for b in range(BPC):
                la = apool.tile([NH, NCH], f32, tag="lacc", name=f"lacc{b}")
                nc.vector.memset(la, 0.0)
                laccs.append(la)

            _acc12 = {}

            def finalize_batch_a(b, acc_lo, acc_hi):
                # DVE phase: l, 1/l, and the group-sum of the 4 col-group acc
                # partials (unscaled). Constraints learned on HW: fp32
                # stationary + row-group tile_position hangs; TensorTensor
                # needs equal base partitions. So: cross-base single-input
                # COPIES to base 0, then aligned adds.
                lacc = laccs[b]
                l_sb = apool.tile([NH, 1], f32, tag="l", name=f"l{b}")
                nc.vector.reduce_sum(out=l_sb, in_=lacc, axis=mybir.AxisListType.X)
                rl_sb = apool.tile([NH, 1], f32, tag="rl", name=f"rl{b}")
                nc.vector.reciprocal(rl_sb, l_sb)
                _rls[b] = rl_sb
                g123_sb = apool.tile([NH, 3, H], f32, tag="g123", name=f"g123{b}")
                for g in (1, 2, 3):
                    sl = slice(32 * g, 32 * g + NH)
                    nc.vector.tensor_copy(g123_sb[:, g - 1, 0:512], acc_lo[sl, :])
                    nc.vector.tensor_copy(g123_sb[:, g - 1, 512:768], acc_hi[sl, :])
                a01_sb = apool.tile([NH, H], f32, tag="a01", name=f"a01{b}")
                nc.vector.tensor_add(out=a01_sb[:, 0:512], in0=acc_lo[0:NH, :],
                                     in1=g123_sb[:, 0, 0:512])
                nc.vector.tensor_add(out=a01_sb[:, 512:768], in0=acc_hi[0:NH, :],
                                     in1=g123_sb[:, 0, 512:768])
                a23_sb = apool.tile([NH, H], f32, tag="a23", name=f"a23{b}")
                nc.vector.tensor_add(out=a23_sb, in0=g123_sb[:, 1, :],
                                     in1=g123_sb[:, 2, :])
                acc12_sb = apool.tile([NH, H], f32, tag="acc12", name=f"acc12{b}")
                nc.vector.tensor_add(out=acc12_sb, in0=a01_sb, in1=a23_sb)
                _acc12[b] = acc12_sb

            def finalize_batch_b(b):
                # PE phase: pooledT (unscaled accT) via base-0 fp32 transposes
                acc12_sb = _acc12[b]
                for j in range(KT):
                    tps = ps_scr.tile([128, NH], f32, tag="pt_scr", bufs=2,
                                      name=f"tps{b}_{j}")
                    nc.tensor.matmul(tps, acc12_sb[:, j * 128:(j + 1) * 128],
                                     id32x4_sb[0:NH, :], start=True, stop=True)
                    nc.vector.tensor_copy(pooledT_sb[:, j, b:2 * NH:2], tps)

            def project_batch_s1(b):
                # stage 1: o_allT = accT_b^T @ w_v^T tiles -> [12 h', 768 hd];
                # the 1/l scaling folds into the PSUM->SBUF copy (rows = h').
                # lo/hi run sequentially so only one pt_scr buffer is held.
                oT_sb = apool.tile([NH, H], f16, tag="oT", name=f"oT{b}")
                for half, (c0, c1) in enumerate(((0, 512), (512, 768))):
                    oT_ps = ps_scr.tile([NH, c1 - c0], f32, tag="pt_scr", bufs=2,
                                        name=f"oT{b}_{half}")
                    for j in range(KT):
                        lhs = pooledT_sb[:, j, b:2 * NH:2]
                        nc.tensor.matmul(oT_ps, lhs, wv_sb[:, j, c0:c1],
                                         start=(j == 0), stop=(j == KT - 1))
                    nc.vector.tensor_scalar_mul(out=oT_sb[:, c0:c1], in0=oT_ps,
                                                scalar1=_rl(b))
                return oT_sb

            def _rl(b):
                return _rls[b]

            def project_batch_s2(b, oT_sb):
                # transpose + diagonal-select: o[hd, b] = o_allT[h'(hd), hd]
                for t in range(KT):
                    ops = ps_scr.tile([128, NH], f32, tag="pt_scr", bufs=2, name=f"ops{b}_{t}")
                    nc.tensor.matmul(ops, oT_sb[:, t * 128:(t + 1) * 128], id16_sb,
                                     start=True, stop=True)
                    nc.vector.tensor_copy(o_sb[t][0:64, b:b + 1],
                                          ops[0:64, 2 * t:2 * t + 1])
                    nc.vector.tensor_copy(o_sb[t][64:128, b:b + 1],
                                          ops[64:128, 2 * t + 1:2 * t + 2])
                # stage 2: out[b, :] = sum_t o_tile_t[:, b]^T @ w_out_g tile
                out_row = apool.tile([1, H], f32, tag="outrow", name=f"outrow{b}")
                for half, (c0, c1) in enumerate(((0, 512), (512, 768))):
                    out_ps = ps_scr.tile([1, c1 - c0], f32, tag="pt_scr", bufs=2,
                                         name=f"out{b}_{half}")
                    for t in range(KT):
                        nc.tensor.matmul(out_ps, o_sb[t][:, b:b + 1],
                                         wog_sb[t][:, c0:c1],
                                         start=(t == 0), stop=(t == KT - 1))
                    nc.vector.tensor_add(out=out_row[:, c0:c1], in0=out_ps,
                                         in1=b2_sb[:, c0:c1])
                nc.gpsimd.dma_start(out=out_d[b:b + 1, :], in_=out_row)

            _accs = {}
            for b in range(BPC):
                acc_lo = ps_acc.tile([128, 512], f32, tag="acc_lo", bufs=2,
                                     name=f"acc_lo{b}")
                acc_hi = ps_acc.tile([128, 256], f32, tag="acc_hi", bufs=2,
                                     name=f"acc_hi{b}")
                _accs[b] = (acc_lo, acc_hi)

                xth_ch = xn_ch = xtt_ch = None
                for ci in range(NCH):
                    dc, oc = divmod(ci, OPC)
                    if oc == 0:
                        # xn streams FIRST, split per 512-sub-chunk (the
                        # transposed chunks consume its early pieces while the
                        # x^T stream for the later chunks still lands); x^T is
                        # split per HBM 512-chunk. First chunk of batch 0
                        # splits xn finer to cut startup latency.
                        nu = DMACHUNK // 128
                        xn_ch = xpool.tile([128, nu, H], f16, tag="xn", bufs=3)
                        xn_in = xn_d[b, dc]
                        nsp = 8 if (b == 0 and dc == 0) else OPC
                        for sp in range(nsp):
                            a0, a1 = sp * nu // nsp, (sp + 1) * nu // nsp
                            nc.sync.dma_start(out=xn_ch[:, a0:a1, :],
                                              in_=xn_in[:, a0:a1, :])
                        if KEEP:
                            # [128, oc', j, s]: each oc' piece is one
                            # contiguous 3KB-per-partition DMA run
                            xth_ch = xpool.tile([128, KEEP, KT, CHUNK], xtdt,
                                                tag="xth", bufs=3)
                            xth_in = xth_d[b, dc]
                            for sp in range(KEEP):
                                nc.sync.dma_start(out=xth_ch[:, sp],
                                                  in_=xth_in[:, sp])
                        if TCH:
                            xtt_ch = xpool.tile([128, KT, TCH * CHUNK], f16,
                                                tag="xtt")

                    hbm = oc >= TCH
                    sig = ps_scr.tile([64, CHUNK], f32, tag="scr", bufs=2)
                    # dependency-gated keep-warm: a tiny matmul reading the
                    # newest DMA piece this 512-chunk needs. It fires when the
                    # piece lands, spreading PE-activity blips across the DMA
                    # wait so the HAM MID window never sees an idle PE. Its
                    # corner of sig is re-zeroed by g0's start=True.
                    kwsrc = (xth_ch[:, oc - TCH, 0, 0:1]
                             if hbm else xn_ch[:, NSUB * oc + NSUB - 1, 0:1])
                    nc.tensor.matmul(sig[0:1, 0:1], kwsrc, kwsrc, start=True,
                                     stop=False, skip_group_check=True)
                    if not hbm:
                        # build this 512-chunk's x^T on-chip: PE transpose of
                        # the natural-layout blocks + PSUM->SBUF copies spread
                        # over DVE / ACT.
                        oc2 = oc
                        for jh in range(2 * KT):
                            j, half = divmod(jh, 2)
                            tp = ps_scr.tile([128, CHUNK // 2], f32,
                                             tag="pt_scr", bufs=2)
                            for t2 in range(NSUB // 2):
                                t = (NSUB // 2) * half + t2
                                u = NSUB * oc + t
                                nc.tensor.matmul(
                                    tp[:, t2 * 128:(t2 + 1) * 128],
                                    xn_ch[:, u, j * 128:(j + 1) * 128],
                                    id128_sb, start=True, stop=True,
                                    skip_group_check=True)
                            c0 = oc2 * CHUNK + half * (CHUNK // 2)
                            dst = xtt_ch[:, j, c0:c0 + CHUNK // 2]
                            if jh % 3 == 2:     # GPSIMD cannot access PSUM
                                nc.scalar.copy(out=dst, in_=tp)
                            else:
                                nc.vector.tensor_copy(dst, tp)

                    # scores: sigma[h, s] over this chunk, 2 col groups
                    if hbm:
                        def rhs(j, och=oc - TCH):
                            return xth_ch[:, och, j, :]
                    else:
                        def rhs(j, oc2=oc):
                            return xtt_ch[:, j, oc2 * CHUNK:(oc2 + 1) * CHUNK]
                    for jj in range(KT // 2):
                        nc.tensor.matmul(sig[0:NH, :], ct_sb[:, jj, :], rhs(jj),
                                         start=(jj == 0), stop=(jj == KT // 2 - 1),
                                         tile_position=(0, 0),
                                         skip_group_check=True)
                        j2 = KT // 2 + jj
                        nc.tensor.matmul(sig[32:32 + NH, :], ct_sb[:, j2, :], rhs(j2),
                                         start=(jj == 0), stop=(jj == KT // 2 - 1),
                                         tile_position=(0, 32),
                                         skip_group_check=True)
                    # p = exp(g0 + g1 - m) = exp(g0 - m) * exp(g1): two ACT
                    # exps straight from PSUM (cross-base reads are fine for
                    # single-input ops), then one cheap fp16 multiply on DVE
                    # whose accum_out collects the l-partial.
                    p0_sb = spool.tile([NH, CHUNK], f16, tag="p0")
                    nc.scalar.activation(out=p0_sb, in_=sig[0:NH, :],
                                         func=mybir.ActivationFunctionType.Exp,
                                         bias=mh_sb[:, b:b + 1], scale=1.0)
                    p1_sb = spool.tile([NH, CHUNK], f16, tag="p1")
                    nc.scalar.activation(out=p1_sb, in_=sig[32:32 + NH, :],
                                         func=mybir.ActivationFunctionType.Exp)
                    p_sb = spool.tile([NH, CHUNK], f16, tag="p")
                    nc.vector.scalar_tensor_tensor(
                        out=p_sb, in0=p0_sb, scalar=1.0, in1=p1_sb,
                        op0=mybir.AluOpType.mult, op1=mybir.AluOpType.mult,
                        accum_out=laccs[b][:, ci:ci + 1])
                    if b == 0 and ci == 4:
                        nc.scalar.dma_start(
                            out=wv_sb,
                            in_=wvt_d.rearrange("(t p) d -> p t d", p=128))
                        for t in range(KT):
                            nc.scalar.dma_start(
                                out=wog_sb[t], in_=wog_d[t * 128:(t + 1) * 128, :])
                        nc.scalar.dma_start(out=b2_sb, in_=b2_d)
                    if b > 0 and ci == 3:
                        _oT[0] = project_batch_s1(b - 1)
                    if b > 0 and ci == 8:
                        project_batch_s2(b - 1, _oT[0])
                    # transpose p -> pT (s on partitions) via identity matmuls
                    pt = ps_scr.tile([128, NSUB * NH + 1], f32, tag="pt_scr", bufs=2)
                    for t in range(NSUB):
                        nc.tensor.matmul(pt[:, t * NH:(t + 1) * NH],
                                         p_sb[:, t * 128:(t + 1) * 128], id16_sb,
                                         start=True, stop=True,
                                         skip_group_check=True)
                    nc.tensor.matmul(pt[0:1, NSUB * NH:], ct_sb[:, 0, 0:1],
                                     ct_sb[:, 0, 0:1], start=True, stop=False,
                                     skip_group_check=True)
                    pT_sb = spool.tile([128, NSUB * NH], f16, tag="pT")
                    nc.vector.tensor_copy(pT_sb, pt[:, :NSUB * NH])
                    # pooled accumulation, 4-col-group packed:
                    # group t accumulates subtile t of every chunk
                    for t in range(NSUB):
                        u = NSUB * oc + t
                        sl = slice(32 * t, 32 * t + NH)
                        nc.tensor.matmul(acc_lo[sl, :], pT_sb[:, t * NH:(t + 1) * NH],
                                         xn_ch[:, u, 0:512],
                                         start=(ci == 0), stop=(ci == NCH - 1),
                                         tile_position=(0, 32 * t),
                                         skip_group_check=True)
                        nc.tensor.matmul(acc_hi[sl, :], pT_sb[:, t * NH:(t + 1) * NH],
                                         xn_ch[:, u, 512:768],
                                         start=(ci == 0), stop=(ci == NCH - 1),
                                         tile_position=(0, 32 * t),
                                         skip_group_check=True)

                    # previous batch's finalize, interleaved into this batch's
                    # first chunks so the PE never sits behind a DVE-only
                    # stretch (which re-throttles the HAM clock gate)
                    if b > 0 and ci == 0:
                        finalize_batch_a(b - 1, *_accs[b - 1])
                    if b > 0 and ci == 1:
                        finalize_batch_b(b - 1)

            finalize_batch_a(BPC - 1, *_accs[BPC - 1])
            finalize_batch_b(BPC - 1)
            project_batch_s2(BPC - 1, project_batch_s1(BPC - 1))

    if split_waits:
        _split_sem_waits(nc, mybir)
    return nc


def _host_prep(x, query, w_kv, b_kv, w_out, b_out, w_gate, b_gate):
    q = query[0, 0].astype(np.float64)
    w_k, w_v = w_kv[:H], w_kv[H:]
    b_v = b_kv[H:]
    scale = 1.0 / np.sqrt(DH)
    C = ((w_k.astype(np.float64).reshape(NH, DH, H) * q.reshape(NH, DH, 1)).sum(1)
         * scale).astype(F32)                                        # (12, 768)
    gate = 1.0 / (1.0 + np.exp(-(q @ w_gate.T.astype(np.float64)
                                 + b_gate.astype(np.float64))))      # (768,)
    w_out_gT = np.ascontiguousarray((gate[:, None] * w_out.astype(np.float64)).T
                                    ).astype(F16)                    # (768hd, 768out)
    bias_full = (gate * (b_out.astype(np.float64)
                         + w_out.astype(np.float64) @ b_v.astype(np.float64))
                 ).astype(F32)                                       # (768,)
    # per-(batch, head) score max for a numerically-safe exp
    sig = (x.reshape(-1, H) @ C.T).reshape(B, S, NH)
    m = sig.max(axis=1)                                              # (B, 12)

    nd = S // DMACHUNK
    # natural layout, pre-tiled: xn[b, dc, p, u, k] = x[b, dc*DMACHUNK+128u+p, k]
    xn16 = np.ascontiguousarray(
        x.reshape(B, nd, DMACHUNK // 128, 128, H)
        .transpose(0, 1, 3, 2, 4)).astype(F16)
    # transposed layout for the HBM-streamed fraction (LAST KEEP*CHUNK s of
    # every DMA chunk; the first TCH*CHUNK are transposed on-chip):
    # xth[b, dc, p, j, s'] = x[b, dc*DMACHUNK+TCH*CHUNK+s', 128j+p]
    xth = None
    if KEEP:
        xt_full = (x.transpose(0, 2, 1).reshape(B, KT, 128, nd, DMACHUNK)
                   .transpose(0, 3, 2, 1, 4))                        # b, dc, p, j, s
        # -> [b, dc, p, oc', j, s'] so each oc' is per-partition contiguous
        xt_keep = (xt_full[..., TCH * CHUNK:]
                   .reshape(B, nd, 128, KT, KEEP, CHUNK)
                   .transpose(0, 1, 2, 4, 3, 5))
        if XT8:
            import ml_dtypes
            xth = np.ascontiguousarray(xt_keep).astype(ml_dtypes.float8_e4m3)
        else:
            xth = np.ascontiguousarray(xt_keep).astype(F16)
    ct16 = np.ascontiguousarray(C.T).astype(F16)                     # (768, 12)
    wvt = np.ascontiguousarray(w_v.T).astype(F16)                    # (768k, 768hd)
    b2 = bias_full.reshape(1, H).copy()
    id32x4 = np.zeros((128, NH), F32)
    for g in range(4):
        id32x4[32 * g:32 * g + NH, :] = np.eye(NH, dtype=F32)

    in_maps = []
    for c in range(NCORES):
        bs = slice(c * BPC, (c + 1) * BPC)
        im = {
            "xn": np.ascontiguousarray(xn16[bs]),
            "ct": ct16,
            "mh": np.ascontiguousarray((-m[bs]).T.astype(F32)),      # (12, BPC)
            "wvt": wvt,
            "wog": w_out_gT,
            "b2": b2,
            "id16": np.eye(NH, dtype=F16),
            "id32x4": id32x4,
            "id128": np.eye(128, dtype=F16),
        }
        if KEEP:
            im["xth"] = np.ascontiguousarray(xth[bs])
        in_maps.append(im)
    return in_maps


_NC_CACHE = {}


def _get_nc():
    if "nc" not in _NC_CACHE:
        _NC_CACHE["nc"] = _build_nc()
    return _NC_CACHE["nc"]


def _install_ntff_shim():
    """Make trace=True work under axon when antenv.axon_hooks is missing."""
    try:
        import antenv.axon_hooks  # noqa: F401
        return
    except ImportError:
        pass
    import antenv
    hooks = types.ModuleType("antenv.axon_hooks")
    hook_box = [None]
    hooks.set_axon_ntff_profile_hook = lambda h: hook_box.__setitem__(0, h)
    hooks.get_axon_ntff_profile_hook = lambda: hook_box[0]
    sys.modules["antenv.axon_hooks"] = hooks
    antenv.axon_hooks = hooks
    so = "/opt/axon/libaxon_pjrt.so"
    if os.path.exists(so):
        try:
            from trn_agent_boot.trn_boot import _ntff_profile_via_ctypes
            hooks.set_axon_ntff_profile_hook(_ntff_profile_via_ctypes(so))
        except Exception:
            pass


def _run(in_maps, trace=False, trace_cores=None):
    from concourse import bass_utils
    if trace:
        _install_ntff_shim()
    nc = _get_nc()
    return bass_utils.run_bass_kernel_spmd(
        nc, in_maps, core_ids=list(range(NCORES)),
        trace=trace, trace_cores=trace_cores)


def kernel(**inputs) -> np.ndarray:
    in_maps = _host_prep(**{k: np.asarray(v) for k, v in inputs.items()})
    res = _run(in_maps, trace=False)
    return np.concatenate([res.results[c]["out"] for c in range(NCORES)], axis=0)


# revision 42
# speedup vs baseline: 1.5710x; 1.5710x over previous
"""AttentivePool (B=16, S=8192, H=768, nH=12, Dh=64, Q=1) for 8 Trainium2 NeuronCores.

Strategy (data-parallel over batch: 2 batches per core):
  Since Q == 1, the K projection collapses to a single 12x768 matrix
  C[h,:] = sum_d q[h,d] * w_k[h*64+d,:] / sqrt(64), so
  scores[b,h,s] = x[b,s,:] . C[h,:]   (b_k adds a per-head constant -> softmax invariant).
  The V/output projections commute with the softmax-weighted sum over s:
  out[b] = w_out_gated @ blockdiag(w_v) @ (attn-weighted mean of x) + const.
  Per batch the device computes:
    sigma = C @ x^T            (PE, needs x^T: k on partitions)
    p     = exp(sigma - m_h)   (ACT; accum_out gives l = sum_s p for free)
    acc   = p^T . x            (PE, needs natural x: s on partitions)
  v2 over the baseline:
    * sigma is 2-col-group packed (j-tiles 0-2 in array cols 0-31, 3-5 in
      32-63, concurrent) with a DVE add combining the two partials.
    * acc is 4-col-group packed: the 4 pooled subtiles of each chunk run
      concurrently in array col groups 0-3, PSUM partitions 32g..32g+12.
    * TCH of every 4 512-chunks get x^T built ON-CHIP from the natural
      stream (PE transpose vs a 128x128 identity + PSUM->SBUF copies on
      DVE/ACT/GPSIMD), cutting the HBM x^T stream by TCH/4.
    * finalize sums the 4 acc groups inside row-tiled accumulating
      transpose matmuls; the 1/l scaling folds into the s1 projection copy.
  Host prep: layout/dtype transforms + exact fold of gate/biases.
"""

import os
import sys
import types

import numpy as np

B, S, H = 16, 8192, 768
NH, DH = 12, 64
NCORES = 8
BPC = B // NCORES          # batches per core
CHUNK = 512                # scores chunk (s columns per PSUM tile)
DMACHUNK = 2048            # DMA granularity in s
NCH = S // CHUNK           # 16 chunks per batch
NSUB = CHUNK // 128        # 4 pooled subtiles per chunk
KT = H // 128              # 6 k-tiles
OPC = DMACHUNK // CHUNK    # 4 512-chunks per DMA chunk

TCH = int(os.environ.get("KERN_TCH", "2"))  # 512-chunks per DMA chunk transposed on-chip (0..4)
KEEP = OPC - TCH           # 512-chunks per DMA chunk streamed as x^T from HBM
XT8 = bool(int(os.environ.get("KERN_XT8", "0")))  # stream the HBM x^T portion as fp8e4m3

F16 = np.float16
F32 = np.float32


def _split_sem_waits(nc, mybir, max_waits=1):
    """walrus codegen rejects >1 semaphore wait per instruction; spread extras
    over preceding same-engine NoOps."""
    for f in nc.m.functions:
        for blk in f.blocks:
            insts = blk.instructions
            new = []
            for inst in insts:
                si = inst.sync_info
                waits = list(si.on_wait) if (si and si.on_wait) else []
                if len(waits) > max_waits:
                    upd = list(si.on_update) if si.on_update else []
                    chunks = [waits[i:i + max_waits] for i in range(0, len(waits), max_waits)]
                    for ci, ch in enumerate(chunks[:-1]):
                        nop = mybir.InstNoOp(name=f"{inst.name}-wsplit{ci}")
                        nop.engine = inst.engine
                        nop.sync_info = mybir.SyncInfo(on_wait=ch, on_update=[])
                        new.append(nop)
                    inst.sync_info = mybir.SyncInfo(on_wait=chunks[-1], on_update=upd)
                new.append(inst)
            blk.instructions = new


def _build_nc(num_devices=NCORES, split_waits=True):
    import concourse.bass as bass
    import concourse.tile as tile
    import concourse.mybir as mybir

    f16 = mybir.dt.float16
    f32 = mybir.dt.float32
    f8 = mybir.dt.float8e4
    xtdt = f8 if XT8 else f16

    nc = bass.Bass("TRN2", target_bir_lowering=False, debug=False,
                   num_devices=num_devices)

    xth_d = xn_d = None
    if KEEP:
        xth_d = nc.dram_tensor("xth", (BPC, S // DMACHUNK, 128, KEEP, KT, CHUNK),
                               xtdt, kind="ExternalInput").ap()
    xn_d = nc.dram_tensor("xn", (BPC, S // DMACHUNK, 128, DMACHUNK // 128, H),
                          f16, kind="ExternalInput").ap()
    ct_d = nc.dram_tensor("ct", (H, NH), f16, kind="ExternalInput").ap()
    mh_d = nc.dram_tensor("mh", (NH, BPC), f32, kind="ExternalInput").ap()
    wvt_d = nc.dram_tensor("wvt", (H, H), f16, kind="ExternalInput").ap()
    wog_d = nc.dram_tensor("wog", (H, H), f16, kind="ExternalInput").ap()
    b2_d = nc.dram_tensor("b2", (1, H), f32, kind="ExternalInput").ap()
    id16_d = nc.dram_tensor("id16", (NH, NH), f16, kind="ExternalInput").ap()
    id32x4_d = nc.dram_tensor("id32x4", (128, NH), f32, kind="ExternalInput").ap()
    id128_d = nc.dram_tensor("id128", (128, 128), f16, kind="ExternalInput").ap()
    out_d = nc.dram_tensor("out", (BPC, H), f32, kind="ExternalOutput").ap()

    with tile.TileContext(nc) as tc:
        with tc.tile_pool(name="consts", bufs=1) as consts, \
             tc.tile_pool(name="xpool", bufs=2) as xpool, \
             tc.tile_pool(name="spool", bufs=4) as spool, \
             tc.tile_pool(name="apool", bufs=2) as apool, \
             tc.tile_pool(name="ps_scr", bufs=2, space="PSUM") as ps_scr, \
             tc.tile_pool(name="ps_tr", bufs=2, space="PSUM") as ps_tr, \
             tc.tile_pool(name="ps_acc", bufs=2, space="PSUM") as ps_acc:

            # ---- load constants (ct first: it gates the first matmul) ----
            ct_sb = consts.tile([128, KT, NH], f16, tag="ct")
            nc.sync.dma_start(out=ct_sb,
                              in_=ct_d.rearrange("(t p) h -> p t h", p=128))
            id16_sb = consts.tile([NH, NH], f16, tag="id16")
            nc.scalar.dma_start(out=id16_sb, in_=id16_d)
            id128_sb = consts.tile([128, 128], f16, tag="id128")
            nc.scalar.dma_start(out=id128_sb, in_=id128_d)
            mh_sb = consts.tile([NH, BPC], f32, tag="mh")
            nc.scalar.dma_start(out=mh_sb, in_=mh_d)
            id32x4_sb = consts.tile([128, NH], f32, tag="id32x4")
            nc.scalar.dma_start(out=id32x4_sb, in_=id32x4_d)

            pooledT_sb = consts.tile([128, KT, 2 * NH], f16, tag="pooledT")  # col = 2h+b per k-tile

            # projection weights: DMA'd mid-way through batch 0 (ACT ring)
            wv_sb = consts.tile([128, KT, H], f16, tag="wv")
            wog_sb = [consts.tile([128, H], f16, tag=f"wog{t}", name=f"wog_sb{t}")
                      for t in range(KT)]
            b2_sb = consts.tile([1, H], f32, tag="b2")
            o_sb = [consts.tile([128, BPC], f16, tag=f"o{t}", name=f"o_sb{t}")
                    for t in range(KT)]

            _oT = [None]
            _rls = {}
            laccs = []
            for b in range(BPC):
                la = apool.tile([NH, NCH], f32, tag="lacc", name=f"lacc{b}")
                nc.vector.memset(la, 0.0)
                laccs.append(la)

            _acc12 = {}

            def finalize_batch_a(b, acc_lo, acc_hi):
                # DVE phase: l, 1/l, and the group-sum of the 4 col-group acc
                # partials (unscaled). Constraints learned on HW: fp32
                # stationary + row-group tile_position hangs; TensorTensor
                # needs equal base partitions. So: cross-base single-input
                # COPIES to base 0, then aligned adds.
                lacc = laccs[b]
                l_sb = apool.tile([NH, 1], f32, tag="l", name=f"l{b}")
                nc.vector.reduce_sum(out=l_sb, in_=lacc, axis=mybir.AxisListType.X)
                rl_sb = apool.tile([NH, 1], f32, tag="rl", name=f"rl{b}")
                nc.vector.reciprocal(rl_sb, l_sb)
                _rls[b] = rl_sb
                g123_sb = apool.tile([NH, 3, H], f32, tag="g123", name=f"g123{b}")
                for g in (1, 2, 3):
                    sl = slice(32 * g, 32 * g + NH)
                    nc.scalar.copy(out=g123_sb[:, g - 1, 0:512], in_=acc_lo[sl, :])
                    nc.scalar.copy(out=g123_sb[:, g - 1, 512:768], in_=acc_hi[sl, :])
                a01_sb = apool.tile([NH, H], f32, tag="a01", name=f"a01{b}")
                nc.vector.tensor_add(out=a01_sb[:, 0:512], in0=acc_lo[0:NH, :],
                                     in1=g123_sb[:, 0, 0:512])
                nc.vector.tensor_add(out=a01_sb[:, 512:768], in0=acc_hi[0:NH, :],
                                     in1=g123_sb[:, 0, 512:768])
                a23_sb = apool.tile([NH, H], f32, tag="a23", name=f"a23{b}")
                nc.vector.tensor_add(out=a23_sb, in0=g123_sb[:, 1, :],
                                     in1=g123_sb[:, 2, :])
                acc12_sb = apool.tile([NH, H], f32, tag="acc12", name=f"acc12{b}")
                nc.vector.tensor_add(out=acc12_sb, in0=a01_sb, in1=a23_sb)
                _acc12[b] = acc12_sb

            def finalize_batch_b(b):
                # PE phase: pooledT (unscaled accT) via base-0 fp32 transposes
                acc12_sb = _acc12[b]
                for j in range(KT):
                    tps = ps_scr.tile([128, NH], f32, tag="pt_scr", bufs=2,
                                      name=f"tps{b}_{j}")
                    nc.tensor.matmul(tps, acc12_sb[:, j * 128:(j + 1) * 128],
                                     id32x4_sb[0:NH, :], start=True, stop=True)
                    nc.vector.tensor_copy(pooledT_sb[:, j, b:2 * NH:2], tps)

            def project_batch_s1(b):
                # stage 1: o_allT = accT_b^T @ w_v^T tiles -> [12 h', 768 hd];
                # the 1/l scaling folds into the PSUM->SBUF copy (rows = h').
                # lo/hi run sequentially so only one pt_scr buffer is held.
                oT_sb = apool.tile([NH, H], f16, tag="oT", name=f"oT{b}")
                for half, (c0, c1) in enumerate(((0, 512), (512, 768))):
                    oT_ps = ps_scr.tile([NH, c1 - c0], f32, tag="pt_scr", bufs=2,
                                        name=f"oT{b}_{half}")
                    for j in range(KT):
                        lhs = pooledT_sb[:, j, b:2 * NH:2]
                        nc.tensor.matmul(oT_ps, lhs, wv_sb[:, j, c0:c1],
                                         start=(j == 0), stop=(j == KT - 1))
                    nc.vector.tensor_scalar_mul(out=oT_sb[:, c0:c1], in0=oT_ps,
                                                scalar1=_rl(b))
                return oT_sb

            def _rl(b):
                return _rls[b]

            def project_batch_s2(b, oT_sb):
                # transpose + diagonal-select: o[hd, b] = o_allT[h'(hd), hd]
                for t in range(KT):
                    ops = ps_scr.tile([128, NH], f32, tag="pt_scr", bufs=2, name=f"ops{b}_{t}")
                    nc.tensor.matmul(ops, oT_sb[:, t * 128:(t + 1) * 128], id16_sb,
                                     start=True, stop=True)
                    nc.vector.tensor_copy(o_sb[t][0:64, b:b + 1],
                                          ops[0:64, 2 * t:2 * t + 1])
                    nc.vector.tensor_copy(o_sb[t][64:128, b:b + 1],
                                          ops[64:128, 2 * t + 1:2 * t + 2])
                # stage 2: out[b, :] = sum_t o_tile_t[:, b]^T @ w_out_g tile
                out_row = apool.tile([1, H], f32, tag="outrow", name=f"outrow{b}")
                for half, (c0, c1) in enumerate(((0, 512), (512, 768))):
                    out_ps = ps_scr.tile([1, c1 - c0], f32, tag="pt_scr", bufs=2,
                                         name=f"out{b}_{half}")
                    for t in range(KT):
                        nc.tensor.matmul(out_ps, o_sb[t][:, b:b + 1],
                                         wog_sb[t][:, c0:c1],
                                         start=(t == 0), stop=(t == KT - 1))
                    nc.vector.tensor_add(out=out_row[:, c0:c1], in0=out_ps,
                                         in1=b2_sb[:, c0:c1])
                nc.gpsimd.dma_start(out=out_d[b:b + 1, :], in_=out_row)

            _accs = {}
            for b in range(BPC):
                acc_lo = ps_acc.tile([128, 512], f32, tag="acc_lo", bufs=1,
                                     name=f"acc_lo{b}")
                acc_hi = ps_acc.tile([128, 256], f32, tag="acc_hi", bufs=1,
                                     name=f"acc_hi{b}")
                _accs[b] = (acc_lo, acc_hi)

                xth_ch = xn_ch = xtt_ch = None
                for ci in range(NCH):
                    dc, oc = divmod(ci, OPC)
                    if oc == 0:
                        # xn streams FIRST, split per 512-sub-chunk (the
                        # transposed chunks consume its early pieces while the
                        # x^T stream for the later chunks still lands); x^T is
                        # split per HBM 512-chunk. First chunk of batch 0
                        # splits xn finer to cut startup latency.
                        nu = DMACHUNK // 128
                        xn_ch = xpool.tile([128, nu, H], f16, tag="xn", bufs=3)
                        xn_in = xn_d[b, dc]
                        nsp = 8 if (b == 0 and dc == 0) else OPC
                        for sp in range(nsp):
                            a0, a1 = sp * nu // nsp, (sp + 1) * nu // nsp
                            nc.sync.dma_start(out=xn_ch[:, a0:a1, :],
                                              in_=xn_in[:, a0:a1, :])
                        if KEEP:
                            # [128, oc', j, s]: each oc' piece is one
                            # contiguous 3KB-per-partition DMA run
                            xth_ch = xpool.tile([128, KEEP, KT, CHUNK], xtdt,
                                                tag="xth", bufs=3)
                            xth_in = xth_d[b, dc]
                            for sp in range(KEEP):
                                nc.sync.dma_start(out=xth_ch[:, sp],
                                                  in_=xth_in[:, sp])
                        if TCH:
                            xtt_ch = xpool.tile([128, KT, TCH * CHUNK], f16,
                                                tag="xtt")

                    hbm = oc >= TCH
                    sig = ps_scr.tile([64, CHUNK], f32, tag="scr", bufs=2)
                    # dependency-gated keep-warm: a tiny matmul reading the
                    # newest DMA piece this 512-chunk needs. It fires when the
                    # piece lands, spreading PE-activity blips across the DMA
                    # wait so the HAM MID window never sees an idle PE. Its
                    # corner of sig is re-zeroed by g0's start=True.
                    kwsrc = (xth_ch[:, oc - TCH, 0, 0:1]
                             if hbm else xn_ch[:, NSUB * oc + NSUB - 1, 0:1])
                    nc.tensor.matmul(sig[0:1, 0:1], kwsrc, kwsrc, start=True,
                                     stop=False, skip_group_check=True)
                    if not hbm:
                        # build this 512-chunk's x^T on-chip: PE transpose of
                        # the natural-layout blocks + PSUM->SBUF copies spread
                        # over DVE / ACT.
                        oc2 = oc
                        for j in range(KT):
                            tp = ps_tr.tile([128, CHUNK], f32, tag="tp",
                                            bufs=2)
                            for t in range(NSUB):
                                u = NSUB * oc + t
                                nc.tensor.matmul(
                                    tp[:, t * 128:(t + 1) * 128],
                                    xn_ch[:, u, j * 128:(j + 1) * 128],
                                    id128_sb, start=True, stop=True,
                                    skip_group_check=True)
                            dst = xtt_ch[:, j, oc2 * CHUNK:(oc2 + 1) * CHUNK]
                            if j in (2, 5):     # GPSIMD cannot access PSUM
                                nc.scalar.copy(out=dst, in_=tp)
                            else:
                                nc.vector.tensor_copy(dst, tp)

                    # scores: sigma[h, s] over this chunk, 2 col groups
                    if hbm:
                        def rhs(j, och=oc - TCH):
                            return xth_ch[:, och, j, :]
                    else:
                        def rhs(j, oc2=oc):
                            return xtt_ch[:, j, oc2 * CHUNK:(oc2 + 1) * CHUNK]
                    for jj in range(KT // 2):
                        nc.tensor.matmul(sig[0:NH, :], ct_sb[:, jj, :], rhs(jj),
                                         start=(jj == 0), stop=(jj == KT // 2 - 1),
                                         tile_position=(0, 0),
                                         skip_group_check=True)
                        j2 = KT // 2 + jj
                        nc.tensor.matmul(sig[32:32 + NH, :], ct_sb[:, j2, :], rhs(j2),
                                         start=(jj == 0), stop=(jj == KT // 2 - 1),
                                         tile_position=(0, 32),
                                         skip_group_check=True)
                    # p = exp(g0 + g1 - m) = exp(g0 - m) * exp(g1): two ACT
                    # exps straight from PSUM (cross-base reads are fine for
                    # single-input ops), then one cheap fp16 multiply on DVE
                    # whose accum_out collects the l-partial.
                    p0_sb = spool.tile([NH, CHUNK], f16, tag="p0")
                    nc.scalar.activation(out=p0_sb, in_=sig[0:NH, :],
                                         func=mybir.ActivationFunctionType.Exp,
                                         bias=mh_sb[:, b:b + 1], scale=1.0)
                    p1_sb = spool.tile([NH, CHUNK], f16, tag="p1")
                    nc.scalar.activation(out=p1_sb, in_=sig[32:32 + NH, :],
                                         func=mybir.ActivationFunctionType.Exp)
                    p_sb = spool.tile([NH, CHUNK], f16, tag="p")
                    nc.vector.scalar_tensor_tensor(
                        out=p_sb, in0=p0_sb, scalar=1.0, in1=p1_sb,
                        op0=mybir.AluOpType.mult, op1=mybir.AluOpType.mult,
                        accum_out=laccs[b][:, ci:ci + 1])
                    if b == 0 and ci == 4:
                        nc.scalar.dma_start(
                            out=wv_sb,
                            in_=wvt_d.rearrange("(t p) d -> p t d", p=128))
                        for t in range(KT):
                            nc.scalar.dma_start(
                                out=wog_sb[t], in_=wog_d[t * 128:(t + 1) * 128, :])
                        nc.scalar.dma_start(out=b2_sb, in_=b2_d)
                    # previous batch's finalize, interleaved here — BEFORE
                    # this chunk's acc matmuls, so the bufs=1 acc PSUM reuse
                    # is ordered read-then-write
                    if b > 0 and ci == 0:
                        finalize_batch_a(b - 1, *_accs[b - 1])
                    if b > 0 and ci == 1:
                        finalize_batch_b(b - 1)
                    if b > 0 and ci == 3:
                        _oT[0] = project_batch_s1(b - 1)
                    if b > 0 and ci == 8:
                        project_batch_s2(b - 1, _oT[0])
                    # transpose p -> pT (s on partitions) via identity matmuls
                    pt = ps_scr.tile([128, NSUB * NH + 1], f32, tag="pt_scr", bufs=2)
                    for t in range(NSUB):
                        nc.tensor.matmul(pt[:, t * NH:(t + 1) * NH],
                                         p_sb[:, t * 128:(t + 1) * 128], id16_sb,
                                         start=True, stop=True,
                                         skip_group_check=True)
                    nc.tensor.matmul(pt[0:1, NSUB * NH:], ct_sb[:, 0, 0:1],
                                     ct_sb[:, 0, 0:1], start=True, stop=False,
                                     skip_group_check=True)
                    pT_sb = spool.tile([128, NSUB * NH], f16, tag="pT")
                    nc.vector.tensor_copy(pT_sb, pt[:, :NSUB * NH])
                    # pooled accumulation, 4-col-group packed:
                    # group t accumulates subtile t of every chunk
                    for t in range(NSUB):
                        u = NSUB * oc + t
                        sl = slice(32 * t, 32 * t + NH)
                        nc.tensor.matmul(acc_lo[sl, :], pT_sb[:, t * NH:(t + 1) * NH],
                                         xn_ch[:, u, 0:512],
                                         start=(ci == 0), stop=(ci == NCH - 1),
                                         tile_position=(0, 32 * t),
                                         skip_group_check=True)
                        nc.tensor.matmul(acc_hi[sl, :], pT_sb[:, t * NH:(t + 1) * NH],
                                         xn_ch[:, u, 512:768],
                                         start=(ci == 0), stop=(ci == NCH - 1),
                                         tile_position=(0, 32 * t),
                                         skip_group_check=True)

            finalize_batch_a(BPC - 1, *_accs[BPC - 1])
            finalize_batch_b(BPC - 1)
            project_batch_s2(BPC - 1, project_batch_s1(BPC - 1))

    if split_waits:
        _split_sem_waits(nc, mybir)
    return nc


def _host_prep(x, query, w_kv, b_kv, w_out, b_out, w_gate, b_gate):
    q = query[0, 0].astype(np.float64)
    w_k, w_v = w_kv[:H], w_kv[H:]
    b_v = b_kv[H:]
    scale = 1.0 / np.sqrt(DH)
    C = ((w_k.astype(np.float64).reshape(NH, DH, H) * q.reshape(NH, DH, 1)).sum(1)
         * scale).astype(F32)                                        # (12, 768)
    gate = 1.0 / (1.0 + np.exp(-(q @ w_gate.T.astype(np.float64)
                                 + b_gate.astype(np.float64))))      # (768,)
    w_out_gT = np.ascontiguousarray((gate[:, None] * w_out.astype(np.float64)).T
                                    ).astype(F16)                    # (768hd, 768out)
    bias_full = (gate * (b_out.astype(np.float64)
                         + w_out.astype(np.float64) @ b_v.astype(np.float64))
                 ).astype(F32)                                       # (768,)
    # per-(batch, head) score max for a numerically-safe exp
    sig = (x.reshape(-1, H) @ C.T).reshape(B, S, NH)
    m = sig.max(axis=1)                                              # (B, 12)

    nd = S // DMACHUNK
    # natural layout, pre-tiled: xn[b, dc, p, u, k] = x[b, dc*DMACHUNK+128u+p, k]
    xn16 = np.ascontiguousarray(
        x.reshape(B, nd, DMACHUNK // 128, 128, H)
        .transpose(0, 1, 3, 2, 4)).astype(F16)
    # transposed layout for the HBM-streamed fraction (LAST KEEP*CHUNK s of
    # every DMA chunk; the first TCH*CHUNK are transposed on-chip):
    # xth[b, dc, p, j, s'] = x[b, dc*DMACHUNK+TCH*CHUNK+s', 128j+p]
    xth = None
    if KEEP:
        xt_full = (x.transpose(0, 2, 1).reshape(B, KT, 128, nd, DMACHUNK)
                   .transpose(0, 3, 2, 1, 4))                        # b, dc, p, j, s
        # -> [b, dc, p, oc', j, s'] so each oc' is per-partition contiguous
        xt_keep = (xt_full[..., TCH * CHUNK:]
                   .reshape(B, nd, 128, KT, KEEP, CHUNK)
                   .transpose(0, 1, 2, 4, 3, 5))
        if XT8:
            import ml_dtypes
            xth = np.ascontiguousarray(xt_keep).astype(ml_dtypes.float8_e4m3)
        else:
            xth = np.ascontiguousarray(xt_keep).astype(F16)
    ct16 = np.ascontiguousarray(C.T).astype(F16)                     # (768, 12)
    wvt = np.ascontiguousarray(w_v.T).astype(F16)                    # (768k, 768hd)
    b2 = bias_full.reshape(1, H).copy()
    id32x4 = np.zeros((128, NH), F32)
    for g in range(4):
        id32x4[32 * g:32 * g + NH, :] = np.eye(NH, dtype=F32)

    in_maps = []
    for c in range(NCORES):
        bs = slice(c * BPC, (c + 1) * BPC)
        im = {
            "xn": np.ascontiguousarray(xn16[bs]),
            "ct": ct16,
            "mh": np.ascontiguousarray((-m[bs]).T.astype(F32)),      # (12, BPC)
            "wvt": wvt,
            "wog": w_out_gT,
            "b2": b2,
            "id16": np.eye(NH, dtype=F16),
            "id32x4": id32x4,
            "id128": np.eye(128, dtype=F16),
        }
        if KEEP:
            im["xth"] = np.ascontiguousarray(xth[bs])
        in_maps.append(im)
    return in_maps


_NC_CACHE = {}


def _get_nc():
    if "nc" not in _NC_CACHE:
        _NC_CACHE["nc"] = _build_nc()
    return _NC_CACHE["nc"]


def _install_ntff_shim():
    """Make trace=True work under axon when antenv.axon_hooks is missing."""
    try:
        import antenv.axon_hooks  # noqa: F401
        return
    except ImportError:
        pass
    import antenv
    hooks = types.ModuleType("antenv.axon_hooks")
    hook_box = [None]
    hooks.set_axon_ntff_profile_hook = lambda h: hook_box.__setitem__(0, h)
    hooks.get_axon_ntff_profile_hook = lambda: hook_box[0]
    sys.modules["antenv.axon_hooks"] = hooks
    antenv.axon_hooks = hooks
    so = "/opt/axon/libaxon_pjrt.so"
    if os.path.exists(so):
        try:
            from trn_agent_boot.trn_boot import _ntff_profile_via_ctypes
            hooks.set_axon_ntff_profile_hook(_ntff_profile_via_ctypes(so))
        except Exception:
            pass


def _run(in_maps, trace=False, trace_cores=None):
    from concourse import bass_utils
    if trace:
        _install_ntff_shim()
    nc = _get_nc()
    return bass_utils.run_bass_kernel_spmd(
        nc, in_maps, core_ids=list(range(NCORES)),
        trace=trace, trace_cores=trace_cores)


def kernel(**inputs) -> np.ndarray:
    in_maps = _host_prep(**{k: np.asarray(v) for k, v in inputs.items()})
    res = _run(in_maps, trace=False)
    return np.concatenate([res.results[c]["out"] for c in range(NCORES)], axis=0)
